# revision 3
# baseline (speedup 1.0000x reference)
"""Trainium2 Bass kernel for nn_CrossAttention (8 heads, head-parallel over 8 cores).

Reference computation (per head h, with N = 64*64 = 4096 tokens, head_dim = 32):
    Q = Wq_h @ X1   [32, N]      (X1 = x1 reshaped to [256, N])
    K = Wk_h @ X2   [32, N]
    V = Wv_h @ X2   [32, N]
    S = Q^T K * scale            [N, N]
    P = softmax(S, axis=-1)
    fused_h = V @ P^T            [32, N]
    out += Wp[:, h] @ fused_h
    out += bp

Per-core device program (core h handles head h):
    - Computes S^T tiles [m, n] on the PE (lhsT = K-block, rhs = Q-chunk),
      exp() on the ScalarE (no max-subtraction needed: scores are O(1)),
      then AV with a ones-row appended to V^T so the softmax denominator
      d[n] = sum_m exp(S[n, m]) falls out of the same accumulation.
    - Projects the *unnormalized* fused values through Wp_h on-device.
    - Returns out_p = Wp_h @ fused_unnorm [256, N] and d [1, N].
Host: out = sum_h out_p_h / d_h + bp  (per-head softmax normalization
commutes with the channel projection, so it can be applied after).
"""

import numpy as np

import concourse.mybir as mybir
import concourse.tile as tile
from concourse import bacc
from concourse.bass_utils import run_bass_kernel_spmd

N_CORES = 8
C = 256          # channels (== hidden dim)
HW = 4096        # tokens (64 * 64)
D = 32           # head dim
SCALE = D ** -0.5
CHUNK = 512      # n-chunk (PSUM bank free size at fp32)
NCHUNKS = HW // CHUNK      # 8
MBLK = 128                 # m-block (PE contraction tile)
NMBLK = HW // MBLK         # 32
GRP = 2                    # m-blocks per exp() instruction group
NGRP = NMBLK // GRP        # 16

FP = mybir.dt.float32
FPR = mybir.dt.float32r    # fp32 storage, full-rate PE streaming


def _emit(nc, tc, x1, x2, wqT, wkT, wvT, wpT, ones, out_p, d_out):
    x1r = x1.rearrange("(k p) n -> p k n", p=128)   # [128, 2, HW]
    x2r = x2.rearrange("(k p) n -> p k n", p=128)
    wqr = wqT.rearrange("(k p) d -> p k d", p=128)  # [128, 2, 32]
    wkr = wkT.rearrange("(k p) d -> p k d", p=128)
    wvr = wvT.rearrange("(k p) d -> p k d", p=128)

    with tc.tile_pool(name="persist", bufs=1) as pp:
        x1_sb = pp.tile([128, 2, HW], FPR)
        x2_sb = pp.tile([128, 2, HW], FPR)
        wq_sb = pp.tile([128, 2, D], FPR)
        wk_sb = pp.tile([128, 2, D], FPR)
        wv_sb = pp.tile([128, 2, D], FPR)
        wp_sb = pp.tile([D, C], FPR)
        q_sb = pp.tile([D, HW], FPR)
        k_sb = pp.tile([D, HW], FPR)
        vt_sb = pp.tile([128, NMBLK, D + 1], FPR)

        nc.sync.dma_start(wq_sb[:], wqr)
        nc.sync.dma_start(wk_sb[:], wkr)
        nc.sync.dma_start(wv_sb[:], wvr)
        nc.sync.dma_start(wp_sb[:], wpT)
        for j in range(NCHUNKS):
            s = slice(j * CHUNK, (j + 1) * CHUNK)
            nc.sync.dma_start(x1_sb[:, :, s], x1r[:, :, s])
            nc.sync.dma_start(x2_sb[:, :, s], x2r[:, :, s])

        # ones row for the denominator trick (DMA'd: memset can't write fp32r)
        nc.sync.dma_start(vt_sb[:, :, D], ones)

        # ---- Q, K: [32, HW] = (WxT)^T @ X ----
        with tc.tile_pool(name="qk_ps", bufs=3, space="PSUM") as qk_ps:
            for j in range(NCHUNKS):
                s = slice(j * CHUNK, (j + 1) * CHUNK)
                for dst, w_sb, src in ((q_sb, wq_sb, x1_sb), (k_sb, wk_sb, x2_sb)):
                    ps = qk_ps.tile([D, CHUNK], FP, tag="qk")
                    nc.tensor.matmul(ps, (w_sb[:, 0, :]), (src[:, 0, s]),
                                     start=True, stop=False)
                    nc.tensor.matmul(ps, (w_sb[:, 1, :]), (src[:, 1, s]),
                                     start=False, stop=True)
                    nc.vector.tensor_copy(dst[:, s], ps)

            # ---- V^T blocks: [128, 32] = (X2-block)^T @ WvT ----
            for t in range(NMBLK):
                b = slice(t * MBLK, (t + 1) * MBLK)
                ps = qk_ps.tile([128, D], FP, tag="v")
                nc.tensor.matmul(ps, (x2_sb[:, 0, b]), (wv_sb[:, 0, :]),
                                 start=True, stop=False)
                nc.tensor.matmul(ps, (x2_sb[:, 1, b]), (wv_sb[:, 1, :]),
                                 start=False, stop=True)
                nc.vector.tensor_copy(vt_sb[:, t, 0:D], ps)

        # ---- attention + projection, per n-chunk ----
        with (
            tc.tile_pool(name="s_ps", bufs=2, space="PSUM") as s_ps,
            tc.tile_pool(name="f_ps", bufs=2, space="PSUM") as f_ps,
            tc.tile_pool(name="o_ps", bufs=2, space="PSUM") as o_ps,
            tc.tile_pool(name="p_sb", bufs=3) as p_pool,
            tc.tile_pool(name="fo_sb", bufs=2) as fo_pool,
        ):
            for j in range(NCHUNKS):
                s = slice(j * CHUNK, (j + 1) * CHUNK)
                fused = f_ps.tile([D + 1, CHUNK], FP, tag="f")
                for g in range(NGRP):
                    s_tile = s_ps.tile([128, GRP * CHUNK], FP, tag="s")
                    for i in range(GRP):
                        t = g * GRP + i
                        nc.tensor.matmul(
                            s_tile[:, i * CHUNK:(i + 1) * CHUNK],
                            (k_sb[:, t * MBLK:(t + 1) * MBLK]),
                            (q_sb[:, s]),
                            start=True, stop=True)
                    p_tile = p_pool.tile([128, GRP * CHUNK], FPR, tag="p")
                    nc.scalar.activation(p_tile, s_tile,
                                         mybir.ActivationFunctionType.Exp,
                                         scale=SCALE)
                    for i in range(GRP):
                        t = g * GRP + i
                        nc.tensor.matmul(
                            fused,
                            (vt_sb[:, t, :]),
                            (p_tile[:, i * CHUNK:(i + 1) * CHUNK]),
                            start=(t == 0), stop=(t == NMBLK - 1))

                f_sb = fo_pool.tile([D + 1, CHUNK], FPR, tag="f")
                nc.vector.tensor_copy(f_sb, fused)
                nc.sync.dma_start(d_out[0:1, s], f_sb[D:D + 1, :])
                for half in range(2):
                    o_tile = o_ps.tile([128, CHUNK], FP, tag="o")
                    nc.tensor.matmul(o_tile,
                                     (wp_sb[:, half * 128:(half + 1) * 128]),
                                     (f_sb[0:D, :]),
                                     start=True, stop=True)
                    o_sb = fo_pool.tile([128, CHUNK], FP, tag="o")
                    nc.vector.tensor_copy(o_sb, o_tile)
                    nc.sync.dma_start(out_p[half * 128:(half + 1) * 128, s], o_sb)


_NC_CACHE = {}


def _get_nc():
    if "nc" not in _NC_CACHE:
        nc = bacc.Bacc("TRN2", target_bir_lowering=False, debug=False,
                       num_devices=N_CORES)
        x1 = nc.dram_tensor("x1", [C, HW], FPR, kind="ExternalInput").ap()
        x2 = nc.dram_tensor("x2", [C, HW], FPR, kind="ExternalInput").ap()
        wqT = nc.dram_tensor("wqT", [C, D], FPR, kind="ExternalInput").ap()
        wkT = nc.dram_tensor("wkT", [C, D], FPR, kind="ExternalInput").ap()
        wvT = nc.dram_tensor("wvT", [C, D], FPR, kind="ExternalInput").ap()
        wpT = nc.dram_tensor("wpT", [D, C], FPR, kind="ExternalInput").ap()
        ones = nc.dram_tensor("ones", [128, NMBLK], FPR, kind="ExternalInput").ap()
        out_p = nc.dram_tensor("out_p", [C, HW], FP, kind="ExternalOutput").ap()
        d_out = nc.dram_tensor("d_out", [1, HW], FPR, kind="ExternalOutput").ap()
        with tile.TileContext(nc) as tc:
            _emit(nc, tc, x1, x2, wqT, wkT, wvT, wpT, ones, out_p, d_out)
        nc.finalize()
        _NC_CACHE["nc"] = nc
    return _NC_CACHE["nc"]


def run(inputs, trace=False, tmpdir=None):
    """inputs: dict with keys x1, x2, Wq, Wk, Wv, Wp, bp (full, unsharded).
    Returns (out [1, C, 64, 64], BassKernelResults)."""
    nc = _get_nc()
    x1 = np.ascontiguousarray(np.asarray(inputs["x1"], np.float32).reshape(C, HW))
    x2 = np.ascontiguousarray(np.asarray(inputs["x2"], np.float32).reshape(C, HW))
    Wq = np.asarray(inputs["Wq"], np.float32)
    Wk = np.asarray(inputs["Wk"], np.float32)
    Wv = np.asarray(inputs["Wv"], np.float32)
    Wp = np.asarray(inputs["Wp"], np.float32)
    bp = np.asarray(inputs["bp"], np.float32)

    in_maps = []
    for h in range(N_CORES):
        sl = slice(D * h, D * (h + 1))
        in_maps.append({
            "x1": x1,
            "x2": x2,
            "wqT": np.ascontiguousarray(Wq[sl, :].T),
            "wkT": np.ascontiguousarray(Wk[sl, :].T),
            "wvT": np.ascontiguousarray(Wv[sl, :].T),
            "wpT": np.ascontiguousarray(Wp[:, sl].T),
            "ones": np.ones((128, NMBLK), np.float32),
        })

    res = run_bass_kernel_spmd(nc, in_maps, core_ids=list(range(N_CORES)),
                               trace=trace, tmpdir=tmpdir)

    acc = np.zeros((C, HW), np.float32)
    for h in range(N_CORES):
        acc += res.results[h]["out_p"] / res.results[h]["d_out"]
    acc += bp[:, None]
    return acc.reshape(1, C, 64, 64), res


def kernel(**inputs):
    out, _ = run(inputs)
    return out
